# revision 22
# baseline (speedup 1.0000x reference)
"""Trainium2 Bass kernel for nn_CausalFlowModel (v2, feature-major pipelined).

Model: encoder MLP -> discretised-LSTM scan over T=1024 -> interpolated
select at per-sample index -> decoder MLP.

Same algebraic trick as v1: feed the scan modified deltas (d for t<idx,
d^2 at t==idx, 0 after) so the final h carry equals the selected /
interpolated value -- no [B,T,Z] materialisation or gather.

v2 layout: everything FEATURE-major ([z-features x batch]) so the
recurrent matmul consumes h directly as PE rhs -- no per-step transpose.
Per core the 64-row batch is split into 2 groups of 32 whose per-step
dependency chains interleave on the engines (software pipelining).

Per step x group: 4 accumulating bf16 matmuls (Whh_g^T h into PSUM
where the Wih^T u_t + b term was precomputed chunk-wise), one sigmoid
over all 4 gate blocks (g-gate weights pre-scaled x2 so tanh(x) =
2*sig(2x)-1, with the 2s-1 fixup on ScalarE), and 6 short fp32 DVE ops
for the cell/output/Euler updates.  State (c,h) and matmul inputs are
bf16 (validated: rel err ~4e-3 vs the 2e-2 gate); all elementwise
intermediates stay fp32.

The per-(batch,t) Euler step size is streamed in as a partition-
broadcast DMA of a host-prepared [1, T*128] row (avoids any on-chip
broadcast work).
"""

import numpy as np
from ml_dtypes import bfloat16 as bf16

import concourse.bass as bass
import concourse.bacc as bacc
import concourse.tile as tile
from concourse import mybir
from concourse.bass_utils import run_bass_kernel_spmd

B, T = 512, 1024
SD, CD = 8, 4
CRS = 64
Z = CRS + SD            # 72
G4 = 4 * Z              # 288
ENC_H = 128
DEC_H = 2 * Z           # 144
OUT = 8
NCORES = 8
BC = B // NCORES        # 64 batch per core
GB = 32                 # batch per group (2 groups)

FP = mybir.dt.float32
BF = mybir.dt.bfloat16
U_CHUNK = 128           # time steps per u-DMA chunk
D_CHUNK = 16            # time steps per d2-DMA chunk
CH = 4                  # scan steps per PSUM gates chunk

# packed-constants column layout: name -> (rows, col_off, cols)
_PACK = {}
_pc = 0
for _name, _r, _c in [
    ("wih", 6, G4), ("whh", Z, G4),
    ("we1", SD, ENC_H), ("we2", ENC_H, ENC_H), ("we3", ENC_H, CRS),
    ("be1", ENC_H, 1), ("be2", ENC_H, 1), ("be3", CRS, 1),
    ("wd1", Z, DEC_H), ("wd2a", 128, DEC_H), ("wd2b", 16, DEC_H),
    ("wd3a", 128, OUT), ("wd3b", 16, OUT),
    ("bd1", 128, 1), ("bd1b", 16, 1), ("bd2", 128, 1), ("bd2b", 16, 1),
    ("bd3", OUT, 1), ("xfm", SD, BC),
]:
    _PACK[_name] = (_r, _pc, _c)
    _pc += _c
PACK_COLS = _pc


def _build_bass():
    nc = bacc.Bacc("TRN2", target_bir_lowering=False, debug=False)

    pack_d = nc.declare_dram_parameter("pack", [128, PACK_COLS], FP,
                                       isOutput=False)
    u_d = nc.declare_dram_parameter("u", [6, T * BC], BF, isOutput=False)
    d2_d = nc.declare_dram_parameter("d2", [1, T * 4 * BC], FP, isOutput=False)
    y_d = nc.declare_dram_parameter("y", [OUT, BC], FP, isOutput=True)

    TANH = mybir.ActivationFunctionType.Tanh
    SIG = mybir.ActivationFunctionType.Sigmoid
    MUL = mybir.AluOpType.mult
    ADD = mybir.AluOpType.add

    with tile.TileContext(nc) as tc:
        with (
            tc.tile_pool(name="w", bufs=1) as wp,
            tc.tile_pool(name="state", bufs=1) as sp,
            tc.tile_pool(name="u", bufs=2) as up,
            tc.tile_pool(name="dd", bufs=2) as dp,
            tc.tile_pool(name="work", bufs=3) as kp,
            tc.tile_pool(name="ps", bufs=1, space="PSUM") as pp,
            tc.tile_pool(name="psg", bufs=3, space="PSUM") as pg,
        ):
            pack = wp.tile([128, PACK_COLS], FP, name="pack_sb", tag="pack_sb")
            nc.gpsimd.dma_start(pack[:], pack_d[:])

            def pk(name):
                r, c0, c = _PACK[name]
                return pack[0:r, c0:c0 + c]

            wih, whh = pk("wih"), pk("whh")
            we1, we2, we3 = pk("we1"), pk("we2"), pk("we3")
            be1, be2, be3 = pk("be1"), pk("be2"), pk("be3")
            wd1, wd2a, wd2b = pk("wd1"), pk("wd2a"), pk("wd2b")
            wd3a, wd3b = pk("wd3a"), pk("wd3b")
            bd1, bd1b, bd2, bd2b, bd3 = (pk("bd1"), pk("bd1b"), pk("bd2"),
                                         pk("bd2b"), pk("bd3"))
            xfm = pk("xfm")

            # bf16 copies of the scan weights
            whh_r = wp.tile([Z, G4], BF, name="whh_r", tag="whh_r")
            nc.vector.tensor_copy(whh_r[:], whh)
            wih_r = wp.tile([6, G4], BF, name="wih_r", tag="wih_r")
            nc.vector.tensor_copy(wih_r[:], wih)

            # ---- persistent per-group state: Bst = [g(32) | c(32) | h(32)]
            Bst = [sp.tile([Z, 5 * GB], BF, name=f"Bst{g}", tag=f"Bst{g}")
                   for g in range(2)]

            # ---- encoder MLP (feature-major) -> z0 ----
            ep1 = pp.tile([ENC_H, BC], FP, tag="mlp")
            nc.tensor.matmul(ep1[:], we1, xfm, start=True, stop=True)
            e1 = kp.tile([ENC_H, BC], FP, tag="enc")
            nc.scalar.activation(e1[:], ep1[:], TANH, bias=be1)
            ep2 = pp.tile([ENC_H, BC], FP, tag="mlp")
            nc.tensor.matmul(ep2[:], we2, e1[:], start=True, stop=True)
            e2 = kp.tile([ENC_H, BC], FP, tag="enc")
            nc.scalar.activation(e2[:], ep2[:], TANH, bias=be2)
            ep3 = pp.tile([CRS, BC], FP, tag="mlp")
            nc.tensor.matmul(ep3[:], we3, e2[:], start=True, stop=True)
            # z0 feature-major, permuted row order [h0 | x]
            z0 = kp.tile([Z, BC], FP, tag="z0")
            nc.vector.tensor_scalar_add(z0[0:CRS, :], ep3[:], be3)
            nc.vector.tensor_copy(z0[CRS:Z, :], xfm)

            for g in range(2):
                nc.vector.memset(Bst[g][:, GB:2 * GB], 0.0)      # c0 = 0
                nc.vector.tensor_copy(Bst[g][:, 2 * GB:3 * GB],
                                      z0[:, g * GB:(g + 1) * GB])

            # ---- the scan ----
            n_chunks = T // CH
            u_cell = [None]
            gp_cell = [None]

            def _precompute(cc):
                cb = cc * CH
                gpn = pg.tile([Z, CH * 256], FP, tag="gp", name="gp")
                uo2 = (cb % U_CHUNK) * BC
                for gi in range(4):
                    nc.tensor.matmul(
                        gpn[:, gi * CH * BC:(gi + 1) * CH * BC],
                        wih_r[:, gi * Z:(gi + 1) * Z],
                        u_cell[0][:, uo2:uo2 + CH * BC],
                        start=(gi % 2 == 0), stop=False,
                        skip_group_check=True)
                gp_cell[0] = gpn
                return gpn

            for ci in range(n_chunks):
                c0 = ci * CH
                if c0 % U_CHUNK == 0:
                    u_r = up.tile([6, U_CHUNK * BC], BF, tag="u")
                    nc.gpsimd.dma_start(
                        u_r[:],
                        u_d[:, c0 * BC:(c0 + U_CHUNK) * BC])
                    u_cell[0] = u_r
                if c0 % D_CHUNK == 0:
                    d2_sb = dp.tile([Z, D_CHUNK * 4 * BC], FP, tag="d2")
                    nc.gpsimd.dma_start(
                        d2_sb[:],
                        d2_d[0:1, c0 * 4 * BC:(c0 + D_CHUNK) * 4 * BC]
                        .to_broadcast([Z, D_CHUNK * 4 * BC]))

                # gates PSUM chunk: cols = gi*CH*64 + tl*64 + gr*32 + b
                # precompute Wih^T u + b for the whole chunk, per gate.
                # start=True clears the whole 2KB PSUM bank, so only the
                # first matmul touching each bank may set it (one gate
                # block = CH*64 fp32 = half a bank at CH=4).
                if ci == 0 or c0 % U_CHUNK == 0:
                    gp = _precompute(ci)
                else:
                    gp = gp_cell[0]

                for tl in range(CH):
                    do = ((c0 + tl) % D_CHUNK) * 4 * BC
                    # phase-split issue order: both groups' MM+sigmoid
                    # first, then both pre-tanh DVE phases, then both
                    # post-tanh phases -- so each engine's FIFO alternates
                    # groups at phase granularity instead of blocking one
                    # group's early ops behind the other's late ops.
                    Sg, Pg = [None, None], [None, None]
                    for gr in range(2):
                        base = tl * BC + gr * GB
                        h_ap = Bst[gr][:, 2 * GB:3 * GB]
                        for gi in range(4):
                            nc.tensor.matmul(
                                gp[:, gi * CH * BC + base:
                                   gi * CH * BC + base + GB],
                                whh_r[:, gi * Z:(gi + 1) * Z],
                                h_ap,
                                start=False, stop=True,
                                skip_group_check=True)
                        S = kp.tile([Z, 4 * GB], FP, tag=f"S{gr}",
                                    name=f"S{gr}")
                        sig_in = gp.rearrange(
                            "p (gi t b) -> p gi (t b)",
                            gi=4, t=CH, b=BC)[:, :, base:base + GB]
                        nc.scalar.activation(S[:], sig_in, SIG)
                        Sg[gr] = S
                    if (tl == 1 and ci + 1 < n_chunks
                            and ((ci + 1) * CH) % U_CHUNK != 0):
                        _precompute(ci + 1)  # fills a PE idle gap mid-chunk
                    for gr in range(2):
                        S = Sg[gr]
                        # g = 2*sig(2x) - 1 = tanh(x)
                        nc.vector.tensor_scalar(
                            Bst[gr][:, 0:GB], S[:, 3 * GB:4 * GB],
                            2.0, -1.0, MUL, ADD)
                        P = kp.tile([Z, 2 * GB], FP, tag=f"P{gr}",
                                    name=f"P{gr}")
                        nc.vector.tensor_mul(P[:], S[:, 0:2 * GB],
                                             Bst[gr][:, 0:2 * GB])
                        # c_cand into the state tile's c' field
                        nc.vector.tensor_add(Bst[gr][:, 3 * GB:4 * GB],
                                             P[:, 0:GB], P[:, GB:2 * GB])
                        TH = kp.tile([Z, GB], FP, tag=f"TH{gr}",
                                     name=f"TH{gr}")
                        nc.scalar.activation(TH[:], Bst[gr][:, 3 * GB:4 * GB],
                                             TANH)
                        Pg[gr] = TH
                    for gr in range(2):
                        S, TH = Sg[gr], Pg[gr]
                        nc.vector.tensor_mul(Bst[gr][:, 4 * GB:5 * GB],
                                             S[:, 2 * GB:3 * GB], TH[:])
                        # Euler blend: [c|h|c'|hc] * [1-d|1-d|d|d], then
                        # pairwise (k, k+64) sum back into [c|h]
                        SC = kp.tile([Z, 4 * GB], FP, tag=f"SC{gr}",
                                     name=f"SC{gr}")
                        nc.vector.tensor_mul(
                            SC[:], Bst[gr][:, GB:5 * GB],
                            d2_sb[:, do + gr * 4 * GB:do + (gr + 1) * 4 * GB])
                        with nc.allow_low_precision("2-term lerp to bf16"):
                            nc.vector.tensor_reduce(
                                Bst[gr][:, GB:3 * GB],
                                SC.rearrange("p (two j) -> p j two", two=2),
                                axis=mybir.AxisListType.X, op=ADD)

            # ---- decoder MLP on final h (feature-major, 2 group slices) ----
            hdec = kp.tile([Z, BC], FP, tag="hdec")
            for g in range(2):
                nc.vector.tensor_copy(hdec[:, g * GB:(g + 1) * GB],
                                      Bst[g][:, 2 * GB:3 * GB])
            dp1 = pp.tile([128, BC], FP, tag="mlp")
            dp1b = pp.tile([16, BC], FP, tag="mlpb")
            for g in range(2):
                h_ap = hdec[:, g * GB:(g + 1) * GB]
                cs = slice(g * GB, (g + 1) * GB)
                nc.tensor.matmul(dp1[:, cs], wd1[:, 0:128], h_ap,
                                 start=True, stop=True)
                nc.tensor.matmul(dp1b[:, cs], wd1[:, 128:DEC_H], h_ap,
                                 start=True, stop=True)
            d1 = kp.tile([128, BC], FP, tag="dec")
            nc.scalar.activation(d1[:], dp1[:], TANH, bias=bd1)
            d1b = kp.tile([16, BC], FP, tag="decb")
            nc.scalar.activation(d1b[:], dp1b[:], TANH, bias=bd1b)

            dp2 = pp.tile([128, BC], FP, tag="mlp")
            nc.tensor.matmul(dp2[:], wd2a[:, 0:128], d1[:], start=True,
                             stop=False)
            nc.tensor.matmul(dp2[:], wd2b[:, 0:128], d1b[:], start=False,
                             stop=True)
            dp2b = pp.tile([16, BC], FP, tag="mlpb")
            nc.tensor.matmul(dp2b[:], wd2a[:, 128:DEC_H], d1[:], start=True,
                             stop=False)
            nc.tensor.matmul(dp2b[:], wd2b[:, 128:DEC_H], d1b[:], start=False,
                             stop=True)
            d2t = kp.tile([128, BC], FP, tag="dec")
            nc.scalar.activation(d2t[:], dp2[:], TANH, bias=bd2)
            d2b = kp.tile([16, BC], FP, tag="decb")
            nc.scalar.activation(d2b[:], dp2b[:], TANH, bias=bd2b)

            dp3 = pp.tile([OUT, BC], FP, tag="mlpb")
            nc.tensor.matmul(dp3[:], wd3a, d2t[:], start=True, stop=False)
            nc.tensor.matmul(dp3[:], wd3b, d2b[:], start=False, stop=True)
            y = kp.tile([OUT, BC], FP, tag="y")
            nc.vector.tensor_scalar_add(y[:], dp3[:], bd3)
            nc.sync.dma_start(y_d[:], y[:])

    nc.compile()
    return nc


_NC_CACHE = None


def _get_nc():
    global _NC_CACHE
    if _NC_CACHE is None:
        _NC_CACHE = _build_bass()
    return _NC_CACHE


def _prep_core_inputs(inputs):
    """Host-side sharding + layout prep. Returns list of per-core in_maps."""
    x = np.asarray(inputs["x"], np.float32)
    rnn = np.asarray(inputs["rnn_input"], np.float32)
    deltas = np.asarray(inputs["deltas"], np.float32)[..., 0]     # [B,T]
    lengths = np.asarray(inputs["lengths"], np.int64)
    idx = np.clip(lengths - 1, 0, T - 1)                          # [B]

    # modified deltas: d for t<idx, d^2 at t==idx, 0 after
    tt = np.arange(T)[None, :]
    dmod = np.where(tt < idx[:, None], deltas,
                    np.where(tt == idx[:, None], deltas * deltas, 0.0)
                    ).astype(np.float32)

    # gate order [i|f|o|g]; within each gate, z order permuted to [h0|x]
    b = (np.asarray(inputs["bih"], np.float32)
         + np.asarray(inputs["bhh"], np.float32))
    perm_z = np.concatenate([np.arange(SD, Z), np.arange(0, SD)])
    gate_perm = np.concatenate([np.arange(0, Z), np.arange(Z, 2 * Z),
                                np.arange(3 * Z, 4 * Z), np.arange(2 * Z, 3 * Z)])
    col_perm = np.concatenate([gate_perm[blk * Z + perm_z] for blk in range(4)])
    wih = np.asarray(inputs["Wih"], np.float32)[:, col_perm]
    whh = np.asarray(inputs["Whh"], np.float32)[np.ix_(perm_z, col_perm)]
    bih_aug = b[col_perm][None, :]                                # [1, 288]
    wih_aug = np.concatenate([wih, bih_aug], axis=0)              # [6, 288]
    # g-gate block scaled x2 so sigmoid(2x) gives (tanh(x)+1)/2
    wih_aug[:, 3 * Z:4 * Z] *= 2.0
    whh = whh.copy()
    whh[:, 3 * Z:4 * Z] *= 2.0

    ones = np.ones((B, T, 1), np.float32)
    u_aug = np.concatenate([rnn, ones], axis=2)                   # [B, T, 6]

    wd2 = np.asarray(inputs["Wd2"], np.float32)
    wd3 = np.asarray(inputs["Wd3"], np.float32)
    wd1p = np.asarray(inputs["Wd1"], np.float32)[perm_z]
    consts = {
        "wih": wih_aug,
        "whh": whh,
        "we1": np.asarray(inputs["We1"], np.float32),
        "we2": np.asarray(inputs["We2"], np.float32),
        "we3": np.asarray(inputs["We3"], np.float32),
        "be1": np.asarray(inputs["be1"], np.float32).reshape(ENC_H, 1),
        "be2": np.asarray(inputs["be2"], np.float32).reshape(ENC_H, 1),
        "be3": np.asarray(inputs["be3"], np.float32).reshape(CRS, 1),
        "wd1": wd1p,
        "wd2a": wd2[0:128],
        "wd2b": wd2[128:DEC_H],
        "wd3a": wd3[0:128],
        "wd3b": wd3[128:DEC_H],
        "bd1": np.asarray(inputs["bd1"], np.float32)[0:128].reshape(128, 1),
        "bd1b": np.asarray(inputs["bd1"], np.float32)[128:].reshape(16, 1),
        "bd2": np.asarray(inputs["bd2"], np.float32)[0:128].reshape(128, 1),
        "bd2b": np.asarray(inputs["bd2"], np.float32)[128:].reshape(16, 1),
        "bd3": np.asarray(inputs["bd3"], np.float32).reshape(OUT, 1),
    }

    base_pack = np.zeros((128, PACK_COLS), np.float32)
    for name, arr in consts.items():
        r, c0, c = _PACK[name]
        assert arr.shape == (r, c), (name, arr.shape, (r, c))
        base_pack[0:r, c0:c0 + c] = arr

    in_maps = []
    for k in range(NCORES):
        rows = slice(k * BC, (k + 1) * BC)
        p = base_pack.copy()
        r, c0, c = _PACK["xfm"]
        p[0:r, c0:c0 + c] = x[rows].T
        dm = dmod[rows]                                           # [64, T]
        # per step, per group: [1-d(32) | 1-d(32) | d(32) | d(32)]
        dA, dB = dm[0:GB], dm[GB:2 * GB]
        d2row = np.concatenate(
            [1.0 - dA, 1.0 - dA, dA, dA, 1.0 - dB, 1.0 - dB, dB, dB],
            axis=0).astype(np.float32)                            # [256, T]
        m = {
            "pack": p,
            "u": np.ascontiguousarray(u_aug[rows].transpose(2, 1, 0)
                                      ).reshape(6, T * BC).astype(bf16),
            "d2": np.ascontiguousarray(d2row.T).reshape(1, T * 4 * BC),
        }
        in_maps.append(m)
    return in_maps


def kernel(**inputs):
    nc = _get_nc()
    in_maps = _prep_core_inputs(inputs)
    res = run_bass_kernel_spmd(nc, in_maps, core_ids=list(range(NCORES)))
    outs = [res.results[k]["y"].T for k in range(NCORES)]   # each [BC, OUT]
    return np.ascontiguousarray(np.concatenate(outs, axis=0).astype(np.float32))


# revision 23
# speedup vs baseline: 1.0271x; 1.0271x over previous
"""Trainium2 Bass kernel for nn_CausalFlowModel (v2, feature-major pipelined).

Model: encoder MLP -> discretised-LSTM scan over T=1024 -> interpolated
select at per-sample index -> decoder MLP.

Same algebraic trick as v1: feed the scan modified deltas (d for t<idx,
d^2 at t==idx, 0 after) so the final h carry equals the selected /
interpolated value -- no [B,T,Z] materialisation or gather.

v2 layout: everything FEATURE-major ([z-features x batch]) so the
recurrent matmul consumes h directly as PE rhs -- no per-step transpose.
Per core the 64-row batch is split into 2 groups of 32 whose per-step
dependency chains interleave on the engines (software pipelining).

Per step x group: 4 accumulating bf16 matmuls (Whh_g^T h into PSUM
where the Wih^T u_t + b term was precomputed chunk-wise), one sigmoid
over all 4 gate blocks (g-gate weights pre-scaled x2 so tanh(x) =
2*sig(2x)-1, with the 2s-1 fixup on ScalarE), and 6 short fp32 DVE ops
for the cell/output/Euler updates.  State (c,h) and matmul inputs are
bf16 (validated: rel err ~4e-3 vs the 2e-2 gate); all elementwise
intermediates stay fp32.

The per-(batch,t) Euler step size is streamed in as a partition-
broadcast DMA of a host-prepared [1, T*128] row (avoids any on-chip
broadcast work).
"""

import numpy as np
from ml_dtypes import bfloat16 as bf16

import concourse.bass as bass
import concourse.bacc as bacc
import concourse.tile as tile
from concourse import mybir
from concourse.bass_utils import run_bass_kernel_spmd

B, T = 512, 1024
SD, CD = 8, 4
CRS = 64
Z = CRS + SD            # 72
G4 = 4 * Z              # 288
ENC_H = 128
DEC_H = 2 * Z           # 144
OUT = 8
NCORES = 8
BC = B // NCORES        # 64 batch per core
GB = 32                 # batch per group (2 groups)

FP = mybir.dt.float32
BF = mybir.dt.bfloat16
U_CHUNK = 128           # time steps per u-DMA chunk
D_CHUNK = 16            # time steps per d2-DMA chunk
CH = 4                  # scan steps per PSUM gates chunk

# packed-constants column layout: name -> (rows, col_off, cols)
_PACK = {}
_pc = 0
for _name, _r, _c in [
    ("wih", 6, G4), ("whh", Z, G4),
    ("we1", SD, ENC_H), ("we2", ENC_H, ENC_H), ("we3", ENC_H, CRS),
    ("be1", ENC_H, 1), ("be2", ENC_H, 1), ("be3", CRS, 1),
    ("wd1", Z, DEC_H), ("wd2a", 128, DEC_H), ("wd2b", 16, DEC_H),
    ("wd3a", 128, OUT), ("wd3b", 16, OUT),
    ("bd1", 128, 1), ("bd1b", 16, 1), ("bd2", 128, 1), ("bd2b", 16, 1),
    ("bd3", OUT, 1), ("xfm", SD, BC),
]:
    _PACK[_name] = (_r, _pc, _c)
    _pc += _c
PACK_COLS = _pc


def _build_bass():
    nc = bacc.Bacc("TRN2", target_bir_lowering=False, debug=False)

    pack_d = nc.declare_dram_parameter("pack", [128, PACK_COLS], FP,
                                       isOutput=False)
    u_d = nc.declare_dram_parameter("u", [6, T * BC], BF, isOutput=False)
    d2_d = nc.declare_dram_parameter("d2", [1, T * 4 * BC], FP, isOutput=False)
    y_d = nc.declare_dram_parameter("y", [OUT, BC], FP, isOutput=True)

    TANH = mybir.ActivationFunctionType.Tanh
    SIG = mybir.ActivationFunctionType.Sigmoid
    MUL = mybir.AluOpType.mult
    ADD = mybir.AluOpType.add

    with tile.TileContext(nc) as tc:
        with (
            tc.tile_pool(name="w", bufs=1) as wp,
            tc.tile_pool(name="state", bufs=1) as sp,
            tc.tile_pool(name="u", bufs=2) as up,
            tc.tile_pool(name="dd", bufs=2) as dp,
            tc.tile_pool(name="work", bufs=3) as kp,
            tc.tile_pool(name="ps", bufs=1, space="PSUM") as pp,
            tc.tile_pool(name="psg", bufs=3, space="PSUM") as pg,
        ):
            pack = wp.tile([128, PACK_COLS], FP, name="pack_sb", tag="pack_sb")
            nc.gpsimd.dma_start(pack[:], pack_d[:])

            def pk(name):
                r, c0, c = _PACK[name]
                return pack[0:r, c0:c0 + c]

            wih, whh = pk("wih"), pk("whh")
            we1, we2, we3 = pk("we1"), pk("we2"), pk("we3")
            be1, be2, be3 = pk("be1"), pk("be2"), pk("be3")
            wd1, wd2a, wd2b = pk("wd1"), pk("wd2a"), pk("wd2b")
            wd3a, wd3b = pk("wd3a"), pk("wd3b")
            bd1, bd1b, bd2, bd2b, bd3 = (pk("bd1"), pk("bd1b"), pk("bd2"),
                                         pk("bd2b"), pk("bd3"))
            xfm = pk("xfm")

            # bf16 copies of the scan weights
            whh_r = wp.tile([Z, G4], BF, name="whh_r", tag="whh_r")
            nc.vector.tensor_copy(whh_r[:], whh)
            wih_r = wp.tile([6, G4], BF, name="wih_r", tag="wih_r")
            nc.vector.tensor_copy(wih_r[:], wih)

            # ---- persistent per-group state: Bst = [g(32) | c(32) | h(32)]
            Bst = [sp.tile([Z, 5 * GB], BF, name=f"Bst{g}", tag=f"Bst{g}")
                   for g in range(2)]

            # ---- encoder MLP (feature-major) -> z0 ----
            ep1 = pp.tile([ENC_H, BC], FP, tag="mlp")
            nc.tensor.matmul(ep1[:], we1, xfm, start=True, stop=True)
            e1 = kp.tile([ENC_H, BC], FP, tag="enc")
            nc.scalar.activation(e1[:], ep1[:], TANH, bias=be1)
            ep2 = pp.tile([ENC_H, BC], FP, tag="mlp")
            nc.tensor.matmul(ep2[:], we2, e1[:], start=True, stop=True)
            e2 = kp.tile([ENC_H, BC], FP, tag="enc")
            nc.scalar.activation(e2[:], ep2[:], TANH, bias=be2)
            ep3 = pp.tile([CRS, BC], FP, tag="mlp")
            nc.tensor.matmul(ep3[:], we3, e2[:], start=True, stop=True)
            # z0 feature-major, permuted row order [h0 | x]
            z0 = kp.tile([Z, BC], FP, tag="z0")
            nc.vector.tensor_scalar_add(z0[0:CRS, :], ep3[:], be3)
            nc.vector.tensor_copy(z0[CRS:Z, :], xfm)

            for g in range(2):
                nc.vector.memset(Bst[g][:, GB:2 * GB], 0.0)      # c0 = 0
                nc.vector.tensor_copy(Bst[g][:, 2 * GB:3 * GB],
                                      z0[:, g * GB:(g + 1) * GB])

            # ---- the scan ----
            n_chunks = T // CH
            u_cell = [None]
            gp_cell = [None]

            def _precompute(cc):
                cb = cc * CH
                gpn = pg.tile([Z, CH * 256], FP, tag="gp", name="gp")
                uo2 = (cb % U_CHUNK) * BC
                for gi in range(4):
                    nc.tensor.matmul(
                        gpn[:, gi * CH * BC:(gi + 1) * CH * BC],
                        wih_r[:, gi * Z:(gi + 1) * Z],
                        u_cell[0][:, uo2:uo2 + CH * BC],
                        start=(gi % 2 == 0), stop=False,
                        skip_group_check=True)
                gp_cell[0] = gpn
                return gpn

            for ci in range(n_chunks):
                c0 = ci * CH
                if c0 % U_CHUNK == 0:
                    u_r = up.tile([6, U_CHUNK * BC], BF, tag="u")
                    nc.gpsimd.dma_start(
                        u_r[:],
                        u_d[:, c0 * BC:(c0 + U_CHUNK) * BC])
                    u_cell[0] = u_r
                if c0 % D_CHUNK == 0:
                    d2_sb = dp.tile([Z, D_CHUNK * 4 * BC], FP, tag="d2")
                    nc.gpsimd.dma_start(
                        d2_sb[:],
                        d2_d[0:1, c0 * 4 * BC:(c0 + D_CHUNK) * 4 * BC]
                        .to_broadcast([Z, D_CHUNK * 4 * BC]))

                # gates PSUM chunk: cols = gi*CH*64 + tl*64 + gr*32 + b
                # precompute Wih^T u + b for the whole chunk, per gate.
                # start=True clears the whole 2KB PSUM bank, so only the
                # first matmul touching each bank may set it (one gate
                # block = CH*64 fp32 = half a bank at CH=4).
                if ci == 0 or c0 % U_CHUNK == 0:
                    gp = _precompute(ci)
                else:
                    gp = gp_cell[0]

                for tl in range(CH):
                    do = ((c0 + tl) % D_CHUNK) * 4 * BC
                    # phase-split issue order: both groups' MM+sigmoid
                    # first, then both pre-tanh DVE phases, then both
                    # post-tanh phases -- so each engine's FIFO alternates
                    # groups at phase granularity instead of blocking one
                    # group's early ops behind the other's late ops.
                    Sg, Pg = [None, None], [None, None]
                    for gr in range(2):
                        base = tl * BC + gr * GB
                        h_ap = Bst[gr][:, 2 * GB:3 * GB]
                        for gi in range(4):
                            nc.tensor.matmul(
                                gp[:, gi * CH * BC + base:
                                   gi * CH * BC + base + GB],
                                whh_r[:, gi * Z:(gi + 1) * Z],
                                h_ap,
                                start=False, stop=True,
                                skip_group_check=True)
                        S = kp.tile([Z, 4 * GB], FP, tag=f"S{gr}",
                                    name=f"S{gr}")
                        sig_in = gp.rearrange(
                            "p (gi t b) -> p gi (t b)",
                            gi=4, t=CH, b=BC)[:, :, base:base + GB]
                        nc.scalar.activation(S[:], sig_in, SIG)
                        Sg[gr] = S
                    if (tl == 1 and ci + 1 < n_chunks
                            and ((ci + 1) * CH) % U_CHUNK != 0):
                        _precompute(ci + 1)  # fills a PE idle gap mid-chunk
                    for gr in range(2):
                        S = Sg[gr]
                        # g = 2*sig(2x) - 1 = tanh(x)
                        nc.vector.tensor_scalar(
                            Bst[gr][:, 0:GB], S[:, 3 * GB:4 * GB],
                            2.0, -1.0, MUL, ADD)
                        P = kp.tile([Z, 2 * GB], FP, tag=f"P{gr}",
                                    name=f"P{gr}")
                        nc.vector.tensor_mul(P[:], S[:, 0:2 * GB],
                                             Bst[gr][:, 0:2 * GB])
                        # c_cand into the state tile's c' field
                        nc.vector.tensor_add(Bst[gr][:, 3 * GB:4 * GB],
                                             P[:, 0:GB], P[:, GB:2 * GB])
                        TH = kp.tile([Z, GB], FP, tag=f"TH{gr}",
                                     name=f"TH{gr}")
                        nc.scalar.activation(TH[:], Bst[gr][:, 3 * GB:4 * GB],
                                             TANH)
                        Pg[gr] = TH
                    for gr in range(2):
                        S, TH = Sg[gr], Pg[gr]
                        nc.vector.tensor_mul(Bst[gr][:, 4 * GB:5 * GB],
                                             S[:, 2 * GB:3 * GB], TH[:])
                        # Euler blend: [c|h|c'|hc] * [1-d|1-d|d|d], then
                        # pairwise (k, k+64) sum back into [c|h]
                        SC = kp.tile([Z, 4 * GB], FP, tag=f"SC{gr}",
                                     name=f"SC{gr}")
                        nc.vector.tensor_mul(
                            SC[:], Bst[gr][:, GB:5 * GB],
                            d2_sb[:, do + gr * 4 * GB:do + (gr + 1) * 4 * GB])
                        # pair-sum = adding the two contiguous scratch
                        # halves; plain TT add is cheaper than TENSOR_REDUCE
                        nc.vector.tensor_add(Bst[gr][:, GB:3 * GB],
                                             SC[:, 0:2 * GB],
                                             SC[:, 2 * GB:4 * GB])

            # ---- decoder MLP on final h (feature-major, 2 group slices) ----
            hdec = kp.tile([Z, BC], FP, tag="hdec")
            for g in range(2):
                nc.vector.tensor_copy(hdec[:, g * GB:(g + 1) * GB],
                                      Bst[g][:, 2 * GB:3 * GB])
            dp1 = pp.tile([128, BC], FP, tag="mlp")
            dp1b = pp.tile([16, BC], FP, tag="mlpb")
            for g in range(2):
                h_ap = hdec[:, g * GB:(g + 1) * GB]
                cs = slice(g * GB, (g + 1) * GB)
                nc.tensor.matmul(dp1[:, cs], wd1[:, 0:128], h_ap,
                                 start=True, stop=True)
                nc.tensor.matmul(dp1b[:, cs], wd1[:, 128:DEC_H], h_ap,
                                 start=True, stop=True)
            d1 = kp.tile([128, BC], FP, tag="dec")
            nc.scalar.activation(d1[:], dp1[:], TANH, bias=bd1)
            d1b = kp.tile([16, BC], FP, tag="decb")
            nc.scalar.activation(d1b[:], dp1b[:], TANH, bias=bd1b)

            dp2 = pp.tile([128, BC], FP, tag="mlp")
            nc.tensor.matmul(dp2[:], wd2a[:, 0:128], d1[:], start=True,
                             stop=False)
            nc.tensor.matmul(dp2[:], wd2b[:, 0:128], d1b[:], start=False,
                             stop=True)
            dp2b = pp.tile([16, BC], FP, tag="mlpb")
            nc.tensor.matmul(dp2b[:], wd2a[:, 128:DEC_H], d1[:], start=True,
                             stop=False)
            nc.tensor.matmul(dp2b[:], wd2b[:, 128:DEC_H], d1b[:], start=False,
                             stop=True)
            d2t = kp.tile([128, BC], FP, tag="dec")
            nc.scalar.activation(d2t[:], dp2[:], TANH, bias=bd2)
            d2b = kp.tile([16, BC], FP, tag="decb")
            nc.scalar.activation(d2b[:], dp2b[:], TANH, bias=bd2b)

            dp3 = pp.tile([OUT, BC], FP, tag="mlpb")
            nc.tensor.matmul(dp3[:], wd3a, d2t[:], start=True, stop=False)
            nc.tensor.matmul(dp3[:], wd3b, d2b[:], start=False, stop=True)
            y = kp.tile([OUT, BC], FP, tag="y")
            nc.vector.tensor_scalar_add(y[:], dp3[:], bd3)
            nc.sync.dma_start(y_d[:], y[:])

    nc.compile()
    return nc


_NC_CACHE = None


def _get_nc():
    global _NC_CACHE
    if _NC_CACHE is None:
        _NC_CACHE = _build_bass()
    return _NC_CACHE


def _prep_core_inputs(inputs):
    """Host-side sharding + layout prep. Returns list of per-core in_maps."""
    x = np.asarray(inputs["x"], np.float32)
    rnn = np.asarray(inputs["rnn_input"], np.float32)
    deltas = np.asarray(inputs["deltas"], np.float32)[..., 0]     # [B,T]
    lengths = np.asarray(inputs["lengths"], np.int64)
    idx = np.clip(lengths - 1, 0, T - 1)                          # [B]

    # modified deltas: d for t<idx, d^2 at t==idx, 0 after
    tt = np.arange(T)[None, :]
    dmod = np.where(tt < idx[:, None], deltas,
                    np.where(tt == idx[:, None], deltas * deltas, 0.0)
                    ).astype(np.float32)

    # gate order [i|f|o|g]; within each gate, z order permuted to [h0|x]
    b = (np.asarray(inputs["bih"], np.float32)
         + np.asarray(inputs["bhh"], np.float32))
    perm_z = np.concatenate([np.arange(SD, Z), np.arange(0, SD)])
    gate_perm = np.concatenate([np.arange(0, Z), np.arange(Z, 2 * Z),
                                np.arange(3 * Z, 4 * Z), np.arange(2 * Z, 3 * Z)])
    col_perm = np.concatenate([gate_perm[blk * Z + perm_z] for blk in range(4)])
    wih = np.asarray(inputs["Wih"], np.float32)[:, col_perm]
    whh = np.asarray(inputs["Whh"], np.float32)[np.ix_(perm_z, col_perm)]
    bih_aug = b[col_perm][None, :]                                # [1, 288]
    wih_aug = np.concatenate([wih, bih_aug], axis=0)              # [6, 288]
    # g-gate block scaled x2 so sigmoid(2x) gives (tanh(x)+1)/2
    wih_aug[:, 3 * Z:4 * Z] *= 2.0
    whh = whh.copy()
    whh[:, 3 * Z:4 * Z] *= 2.0

    ones = np.ones((B, T, 1), np.float32)
    u_aug = np.concatenate([rnn, ones], axis=2)                   # [B, T, 6]

    wd2 = np.asarray(inputs["Wd2"], np.float32)
    wd3 = np.asarray(inputs["Wd3"], np.float32)
    wd1p = np.asarray(inputs["Wd1"], np.float32)[perm_z]
    consts = {
        "wih": wih_aug,
        "whh": whh,
        "we1": np.asarray(inputs["We1"], np.float32),
        "we2": np.asarray(inputs["We2"], np.float32),
        "we3": np.asarray(inputs["We3"], np.float32),
        "be1": np.asarray(inputs["be1"], np.float32).reshape(ENC_H, 1),
        "be2": np.asarray(inputs["be2"], np.float32).reshape(ENC_H, 1),
        "be3": np.asarray(inputs["be3"], np.float32).reshape(CRS, 1),
        "wd1": wd1p,
        "wd2a": wd2[0:128],
        "wd2b": wd2[128:DEC_H],
        "wd3a": wd3[0:128],
        "wd3b": wd3[128:DEC_H],
        "bd1": np.asarray(inputs["bd1"], np.float32)[0:128].reshape(128, 1),
        "bd1b": np.asarray(inputs["bd1"], np.float32)[128:].reshape(16, 1),
        "bd2": np.asarray(inputs["bd2"], np.float32)[0:128].reshape(128, 1),
        "bd2b": np.asarray(inputs["bd2"], np.float32)[128:].reshape(16, 1),
        "bd3": np.asarray(inputs["bd3"], np.float32).reshape(OUT, 1),
    }

    base_pack = np.zeros((128, PACK_COLS), np.float32)
    for name, arr in consts.items():
        r, c0, c = _PACK[name]
        assert arr.shape == (r, c), (name, arr.shape, (r, c))
        base_pack[0:r, c0:c0 + c] = arr

    in_maps = []
    for k in range(NCORES):
        rows = slice(k * BC, (k + 1) * BC)
        p = base_pack.copy()
        r, c0, c = _PACK["xfm"]
        p[0:r, c0:c0 + c] = x[rows].T
        dm = dmod[rows]                                           # [64, T]
        # per step, per group: [1-d(32) | 1-d(32) | d(32) | d(32)]
        dA, dB = dm[0:GB], dm[GB:2 * GB]
        d2row = np.concatenate(
            [1.0 - dA, 1.0 - dA, dA, dA, 1.0 - dB, 1.0 - dB, dB, dB],
            axis=0).astype(np.float32)                            # [256, T]
        m = {
            "pack": p,
            "u": np.ascontiguousarray(u_aug[rows].transpose(2, 1, 0)
                                      ).reshape(6, T * BC).astype(bf16),
            "d2": np.ascontiguousarray(d2row.T).reshape(1, T * 4 * BC),
        }
        in_maps.append(m)
    return in_maps


def kernel(**inputs):
    nc = _get_nc()
    in_maps = _prep_core_inputs(inputs)
    res = run_bass_kernel_spmd(nc, in_maps, core_ids=list(range(NCORES)))
    outs = [res.results[k]["y"].T for k in range(NCORES)]   # each [BC, OUT]
    return np.ascontiguousarray(np.concatenate(outs, axis=0).astype(np.float32))
